# revision 21
# baseline (speedup 1.0000x reference)
"""DiceBCELossWithTopology fused loss kernel for Trainium2 (8 NeuronCores).

Reference computation (on inputs x, t of shape (64,1,512,512) f32, flattened):
  dice  = 1 - (2*sum(x*t)+1) / (sum(x)+sum(t)+1)
  bce   = mean(-(t*max(log x,-100) + (1-t)*max(log1p(-x),-100)))
  topo  = |n_runs_of_nonzero(x) - 1| / (512*512)
  loss  = 0.5*bce + dice + topo

Strategy (data-parallel over 8 cores, memory-bound):
  Each core gets a contiguous 2M-element shard viewed as [128, 16384],
  streamed in 8 chunks of [128, 2048] (triple-buffered).  Per chunk:
    ACT : L1 = Ln(x), L2 = Ln(1-x) (accum_out -> free sum(L2)); both
          write interleaved sections of one wide rhs tile R.
    DVE : xb = bf16(x) via tensor_scalar mult 1.0 (accum_out -> free
          sum(x)), tb = bf16(t), bz = (x==0), starts = (bz_prev>bz_cur),
          clamp L1 to -100 in place.
    PE  : ONE wide matmul per 128-col sub-chunk with lhsT = tb-cols and
          rhs = [L1c | ones | L2 | xb] (385 cols), PSUM-accumulated
          into ping-pong banks: diagonals give sum(t*L1c), sum(t*L2),
          sum(x*t); the ones column gives sum(t).  Plus ones-weight
          matmuls over `starts` for the run-start count.
  Host: float64 final reduction over tiny per-core stats + row/shard
  boundary run-start corrections (1031 element pairs) + loss assembly.

log(1-x) never needs clamping: 1-x is exact in f32 and >= 2^-24 for
x in [0,1), so log1p(-x) >= -17.  log(x) hits -inf only at x == 0; the
DVE max(L1, -100) clamp maps -inf -> -100 exactly (verified on HW).
bf16(x) == 0 iff x == 0 for this input domain, so topology is exact.
"""

import numpy as np

# Problem constants (hardcoded per harness contract - no file reads here).
N_CORES = 8
P = 128                      # SBUF partitions
COLS = 16384                 # columns per core: 2M elements / 128
# Chunk widths: big chunks for streaming, tapered tail so the last
# chunk's serial DMA->ACT->DVE->PE chain is short.
CHUNKS = [2048] * 7 + [1024, 512, 512]
NCHUNK = len(CHUNKS)
SUB = 128                    # matmul sub-chunk width (weight columns)
TOTAL = 64 * 512 * 512       # 16_777_216 elements
IMAGE_PIXELS = 512 * 512
SMOOTH = 1.0
LOG_CLAMP = -100.0
BCE_WEIGHT = 0.5
TOPOLOGY_WEIGHT = 1.0

# rhs group layout: [L1c 0:128 | ones 128 | L2 129:257 | xb 257:385]
GW = 388                     # group stride (padded to even)
NRHS = 385                   # matmul free size

_CACHE = {}


def _build_nc():
    from concourse.bacc import Bacc
    import concourse.mybir as mybir
    from concourse.tile import TileContext

    F32 = mybir.dt.float32
    BF16 = mybir.dt.bfloat16
    AF = mybir.ActivationFunctionType
    OP = mybir.AluOpType
    AX = mybir.AxisListType

    nc = Bacc()
    x_d = nc.dram_tensor("x", [P, COLS], F32, kind="ExternalInput")
    t_d = nc.dram_tensor("t", [P, COLS], F32, kind="ExternalInput")
    eye_d = nc.dram_tensor("eye", [P, SUB], F32, kind="ExternalInput")
    stats_d = nc.dram_tensor("stats", [P, 32], F32, kind="ExternalOutput")

    with TileContext(nc) as tc:
        with tc.tile_pool(name="const", bufs=1) as cpool, \
             tc.tile_pool(name="work", bufs=5) as pool, \
             tc.tile_pool(name="psum", bufs=1, space="PSUM") as psum_pool:

            eye = cpool.tile([P, SUB], F32)
            onesW = cpool.tile([P, SUB], BF16)
            stats = cpool.tile([P, 32], F32)

            # Two PSUM banks per accumulation stream (ping-pong): matmul N
            # into bank (N%2) overlaps its drain with matmul N+1's fill.
            psumB = [psum_pool.tile([P, NRHS], F32, name=f"psumB{i}")
                     for i in range(2)]              # fused dots + sum(t)
            psumX = [psum_pool.tile([P, 512], F32, name=f"psumX{i}")
                     for i in range(2)]              # sum(x) colsums

            FCMAX = max(CHUNKS)
            off = 0
            for j, FC in enumerate(CHUNKS):
                NSUB = FC // SUB
                x_t = pool.tile([P, FCMAX + 1], F32, tag="x_t", name=f"x_t{j}")[:, :FC + 1]
                t_t = pool.tile([P, FCMAX], F32, tag="t_t", name=f"t_t{j}")[:, :FC]
                tb = pool.tile([P, FCMAX], BF16, tag="tb", name=f"tb{j}")[:, :FC]
                R = pool.tile([P, (FCMAX // SUB) * GW], BF16,
                              tag="R", name=f"R{j}")[:, :NSUB * GW]
                st = pool.tile([P, FCMAX], BF16, tag="st", name=f"st{j}")[:, :FC]

                # ---- DMA in (overlap col 0 = previous element of same row)
                if j == 0:
                    nc.sync.dma_start(x_t[:, 1:FC + 1], x_d[:, 0:FC])
                    nc.vector.memset(x_t[:, 0:1], 1.0)  # no phantom run start
                else:
                    nc.sync.dma_start(x_t, x_d[:, off - 1:off + FC])
                nc.sync.dma_start(t_t, t_d[:, off:off + FC])

                if j == 0:
                    # const setup - after the first DMAs so they issue first
                    nc.sync.dma_start(eye[:], eye_d[:])
                    nc.vector.memset(onesW[:], 1.0)
                    nc.vector.memset(stats[:], 0.0)

                x3 = x_t[:, 1:FC + 1].rearrange("p (g w) -> p g w", w=SUB)
                R3 = R.rearrange("p (g w) -> p g w", w=GW)

                # ---- ACT: logs (bf16 out) with free accumulation of sum(L2)
                nc.scalar.activation(R3[:, :, 0:SUB], x3, AF.Ln)
                nc.scalar.activation(R3[:, :, SUB + 1:2 * SUB + 1], x3, AF.Ln,
                                     scale=-1.0, bias=1.0,
                                     accum_out=stats[:, 8 + j:9 + j])

                # ---- DVE: casts, fused run-start detect+count, clamp
                nc.vector.tensor_copy(R3[:, :, 2 * SUB + 1:3 * SUB + 1], x3)
                nc.vector.tensor_copy(tb, t_t)
                nc.vector.memset(R3[:, :, SUB:SUB + 1], 1.0)
                # starts = (x_prev == 0) & (x_cur != 0), summed for free
                nc.vector.scalar_tensor_tensor(
                    out=st, in0=x_t[:, 0:FC], scalar=0.0,
                    in1=x_t[:, 1:FC + 1], op0=OP.is_equal, op1=OP.logical_and,
                    accum_out=stats[:, 20 + j:21 + j])
                nc.vector.tensor_scalar(R3[:, :, 0:SUB], R3[:, :, 0:SUB],
                                        LOG_CLAMP, None, OP.max)

                # ---- PE: one wide fused matmul per sub-chunk + sum(x)
                for c in range(NSUB):
                    first = (j == 0 and c < 2)
                    last = (j == NCHUNK - 1 and c >= NSUB - 2)
                    nc.tensor.matmul(
                        psumB[c % 2][:], tb[:, c * SUB:(c + 1) * SUB],
                        R[:, c * GW:c * GW + NRHS],
                        start=first, stop=last, skip_group_check=True)
                ng = max(NSUB // 4, 1)
                for s in range(ng):
                    first = (j == 0 and s < 2)
                    last = (j == NCHUNK - 1 and s >= ng - 2)
                    g0, g1 = 4 * s, min(4 * s + 4, NSUB)
                    nc.tensor.matmul(
                        psumX[s % 2][:, 0:(g1 - g0) * SUB], onesW[:],
                        R3[:, g0:g1, 2 * SUB + 1:3 * SUB + 1],
                        start=first, stop=last, skip_group_check=True)
                off += FC

            # ---- extraction: PSUM -> stats columns
            psB_sb = cpool.tile([P, NRHS], F32)
            scr = cpool.tile([P, SUB], F32)
            nc.vector.tensor_reduce(stats[:, 5:6], psumX[0][:], AX.X, OP.add)
            nc.vector.tensor_reduce(stats[:, 6:7], psumX[1][:], AX.X, OP.add)
            nc.scalar.copy(psB_sb[:], psumB[0][:])
            nc.vector.tensor_tensor(psB_sb[:], psB_sb[:], psumB[1][:], OP.add)
            nc.vector.tensor_tensor(scr[:], psB_sb[:, 0:SUB], eye[:], OP.mult)
            nc.vector.tensor_reduce(stats[:, 0:1], scr[:], AX.X, OP.add)   # t.L1c
            nc.vector.tensor_copy(stats[:, 1:2], psB_sb[:, SUB:SUB + 1])   # sum t
            nc.vector.tensor_tensor(scr[:], psB_sb[:, SUB + 1:2 * SUB + 1],
                                    eye[:], OP.mult)
            nc.vector.tensor_reduce(stats[:, 2:3], scr[:], AX.X, OP.add)   # t.L2
            nc.vector.tensor_tensor(scr[:], psB_sb[:, 2 * SUB + 1:3 * SUB + 1],
                                    eye[:], OP.mult)
            nc.vector.tensor_reduce(stats[:, 3:4], scr[:], AX.X, OP.add)   # x.t
            nc.sync.dma_start(stats_d[:], stats[:])

    nc.finalize()
    return nc


def _get_nc():
    if "nc" not in _CACHE:
        _CACHE["nc"] = _build_nc()
    return _CACHE["nc"]


def kernel(inputs: np.ndarray, targets: np.ndarray) -> np.ndarray:
    from concourse.bass_utils import run_bass_kernel_spmd

    xf = np.ascontiguousarray(inputs, dtype=np.float32).reshape(-1)
    tf = np.ascontiguousarray(targets, dtype=np.float32).reshape(-1)
    assert xf.size == TOTAL and tf.size == TOTAL

    eye = np.eye(P, SUB, dtype=np.float32)
    shard = TOTAL // N_CORES
    in_maps = []
    for c in range(N_CORES):
        in_maps.append({
            "x": xf[c * shard:(c + 1) * shard].reshape(P, COLS),
            "t": tf[c * shard:(c + 1) * shard].reshape(P, COLS),
            "eye": eye,
        })

    nc = _get_nc()
    res = run_bass_kernel_spmd(nc, in_maps, core_ids=list(range(N_CORES)))

    s_xt = s_x = s_t = t1 = t2 = s_l2 = 0.0
    n_starts = 0.0
    for c in range(N_CORES):
        stt = res.results[c]["stats"].astype(np.float64)
        t1 += stt[:, 0].sum()
        s_t += stt[:, 1].sum()
        t2 += stt[:, 2].sum()
        s_xt += stt[:, 3].sum()
        s_x += stt[0, 5] + stt[0, 6]       # each row already holds bank totals
        s_l2 += stt[:, 8:8 + NCHUNK].sum()
        n_starts += stt[:, 20:20 + NCHUNK].sum()

    # Host-side boundary run starts: row boundaries (incl. shard cuts) and
    # the first element.  1023 pairs + 1 element - O(1) work.
    prev = xf[COLS - 1:-1:COLS]
    cur = xf[COLS::COLS]
    n_starts += np.count_nonzero((cur != 0) & (prev == 0))
    n_starts += float(xf[0] != 0)

    dice = 1.0 - (2.0 * s_xt + SMOOTH) / (s_x + s_t + SMOOTH)
    bce = -(t1 - t2 + s_l2) / TOTAL
    topo = abs(n_starts - 1.0) / IMAGE_PIXELS
    loss = bce * BCE_WEIGHT + dice + topo * TOPOLOGY_WEIGHT
    return np.array(loss, dtype=np.float32)


# revision 22
# speedup vs baseline: 1.0847x; 1.0847x over previous
"""DiceBCELossWithTopology fused loss kernel for Trainium2 (8 NeuronCores).

Reference computation (on inputs x, t of shape (64,1,512,512) f32, flattened):
  dice  = 1 - (2*sum(x*t)+1) / (sum(x)+sum(t)+1)
  bce   = mean(-(t*max(log x,-100) + (1-t)*max(log1p(-x),-100)))
  topo  = |n_runs_of_nonzero(x) - 1| / (512*512)
  loss  = 0.5*bce + dice + topo

Strategy (data-parallel over 8 cores, memory-bound):
  Each core gets a contiguous 2M-element shard viewed as [128, 16384],
  streamed in 8 chunks of [128, 2048] (triple-buffered).  Per chunk:
    ACT : L1 = Ln(x), L2 = Ln(1-x) (accum_out -> free sum(L2)); both
          write interleaved sections of one wide rhs tile R.
    DVE : xb = bf16(x) via tensor_scalar mult 1.0 (accum_out -> free
          sum(x)), tb = bf16(t), bz = (x==0), starts = (bz_prev>bz_cur),
          clamp L1 to -100 in place.
    PE  : ONE wide matmul per 128-col sub-chunk with lhsT = tb-cols and
          rhs = [L1c | ones | L2 | xb] (385 cols), PSUM-accumulated
          into ping-pong banks: diagonals give sum(t*L1c), sum(t*L2),
          sum(x*t); the ones column gives sum(t).  Plus ones-weight
          matmuls over `starts` for the run-start count.
  Host: float64 final reduction over tiny per-core stats + row/shard
  boundary run-start corrections (1031 element pairs) + loss assembly.

log(1-x) never needs clamping: 1-x is exact in f32 and >= 2^-24 for
x in [0,1), so log1p(-x) >= -17.  log(x) hits -inf only at x == 0; the
DVE max(L1, -100) clamp maps -inf -> -100 exactly (verified on HW).
bf16(x) == 0 iff x == 0 for this input domain, so topology is exact.
"""

import numpy as np

# Problem constants (hardcoded per harness contract - no file reads here).
N_CORES = 8
P = 128                      # SBUF partitions
COLS = 16384                 # columns per core: 2M elements / 128
# Chunk widths: big chunks for streaming, tapered tail so the last
# chunk's serial DMA->ACT->DVE->PE chain is short.
CHUNKS = [2048] * 7 + [1024, 512, 512]
NCHUNK = len(CHUNKS)
SUB = 128                    # matmul sub-chunk width (weight columns)
TOTAL = 64 * 512 * 512       # 16_777_216 elements
IMAGE_PIXELS = 512 * 512
SMOOTH = 1.0
LOG_CLAMP = -100.0
BCE_WEIGHT = 0.5
TOPOLOGY_WEIGHT = 1.0

# rhs group layout: [L1c 0:128 | ones 128 | L2 129:257 | xb 257:385]
GW = 388                     # group stride (padded to even)
NRHS = 385                   # matmul free size

_CACHE = {}


def _build_nc():
    from concourse.bacc import Bacc
    import concourse.mybir as mybir
    from concourse.tile import TileContext

    F32 = mybir.dt.float32
    BF16 = mybir.dt.bfloat16
    AF = mybir.ActivationFunctionType
    OP = mybir.AluOpType
    AX = mybir.AxisListType

    nc = Bacc()
    x_d = nc.dram_tensor("x", [P, COLS], F32, kind="ExternalInput")
    t_d = nc.dram_tensor("t", [P, COLS], F32, kind="ExternalInput")
    eye_d = nc.dram_tensor("eye", [P, SUB], F32, kind="ExternalInput")
    stats_d = nc.dram_tensor("stats", [P, 32], F32, kind="ExternalOutput")

    with TileContext(nc) as tc:
        with tc.tile_pool(name="const", bufs=1) as cpool, \
             tc.tile_pool(name="work", bufs=4) as pool, \
             tc.tile_pool(name="psum", bufs=1, space="PSUM") as psum_pool:

            eye = cpool.tile([P, SUB], F32)
            onesW = cpool.tile([P, SUB], BF16)
            stats = cpool.tile([P, 32], F32)

            # Two PSUM banks per accumulation stream (ping-pong): matmul N
            # into bank (N%2) overlaps its drain with matmul N+1's fill.
            psumB = [psum_pool.tile([P, NRHS], F32, name=f"psumB{i}")
                     for i in range(2)]              # fused dots + sum(t)
            psumX = [psum_pool.tile([P, 512], F32, name=f"psumX{i}")
                     for i in range(2)]              # sum(x) colsums

            FCMAX = max(CHUNKS)
            off = 0
            for j, FC in enumerate(CHUNKS):
                NSUB = FC // SUB
                x_t = pool.tile([P, FCMAX + 1], F32, tag="x_t", name=f"x_t{j}")[:, :FC + 1]
                t_t = pool.tile([P, FCMAX], F32, tag="t_t", name=f"t_t{j}")[:, :FC]
                tb = pool.tile([P, FCMAX], BF16, tag="tb", name=f"tb{j}")[:, :FC]
                R = pool.tile([P, (FCMAX // SUB) * GW], BF16,
                              tag="R", name=f"R{j}")[:, :NSUB * GW]
                st = pool.tile([P, FCMAX], BF16, tag="st", name=f"st{j}")[:, :FC]

                # ---- DMA in (overlap col 0 = previous element of same row)
                if j == 0:
                    nc.sync.dma_start(x_t[:, 1:FC + 1], x_d[:, 0:FC])
                    nc.vector.memset(x_t[:, 0:1], 1.0)  # no phantom run start
                else:
                    nc.sync.dma_start(x_t, x_d[:, off - 1:off + FC])
                nc.sync.dma_start(t_t, t_d[:, off:off + FC])

                if j == 0:
                    # const setup - after the first DMAs so they issue first
                    nc.sync.dma_start(eye[:], eye_d[:])
                    nc.vector.memset(onesW[:], 1.0)
                    nc.vector.memset(stats[:], 0.0)

                x3 = x_t[:, 1:FC + 1].rearrange("p (g w) -> p g w", w=SUB)
                R3 = R.rearrange("p (g w) -> p g w", w=GW)

                # ---- ACT: logs (bf16 out) with free accumulation of sum(L2)
                nc.scalar.activation(R3[:, :, 0:SUB], x3, AF.Ln)
                nc.scalar.activation(R3[:, :, SUB + 1:2 * SUB + 1], x3, AF.Ln,
                                     scale=-1.0, bias=1.0,
                                     accum_out=stats[:, 8 + j:9 + j])

                # ---- DVE: casts, fused run-start detect+count, clamp
                nc.vector.tensor_copy(R3[:, :, 2 * SUB + 1:3 * SUB + 1], x3)
                nc.vector.tensor_copy(tb, t_t)
                nc.vector.memset(R3[:, :, SUB:SUB + 1], 1.0)
                # starts = (x_prev == 0) & (x_cur != 0), summed for free
                nc.vector.scalar_tensor_tensor(
                    out=st, in0=x_t[:, 0:FC], scalar=0.0,
                    in1=x_t[:, 1:FC + 1], op0=OP.is_equal, op1=OP.logical_and,
                    accum_out=stats[:, 20 + j:21 + j])
                nc.vector.tensor_scalar(R3[:, :, 0:SUB], R3[:, :, 0:SUB],
                                        LOG_CLAMP, None, OP.max)

                # ---- PE: one wide fused matmul per sub-chunk + sum(x)
                for c in range(NSUB):
                    first = (j == 0 and c < 2)
                    last = (j == NCHUNK - 1 and c >= NSUB - 2)
                    nc.tensor.matmul(
                        psumB[c % 2][:], tb[:, c * SUB:(c + 1) * SUB],
                        R[:, c * GW:c * GW + NRHS],
                        start=first, stop=last, skip_group_check=True)
                ng = max(NSUB // 4, 1)
                for s in range(ng):
                    first = (j == 0 and s < 2)
                    last = (j == NCHUNK - 1 and s >= ng - 2)
                    g0, g1 = 4 * s, min(4 * s + 4, NSUB)
                    nc.tensor.matmul(
                        psumX[s % 2][:, 0:(g1 - g0) * SUB], onesW[:],
                        R3[:, g0:g1, 2 * SUB + 1:3 * SUB + 1],
                        start=first, stop=last, skip_group_check=True)
                off += FC

            # ---- extraction: PSUM -> stats columns
            psB_sb = cpool.tile([P, NRHS], F32)
            scr = cpool.tile([P, SUB], F32)
            nc.vector.tensor_reduce(stats[:, 5:6], psumX[0][:], AX.X, OP.add)
            nc.vector.tensor_reduce(stats[:, 6:7], psumX[1][:], AX.X, OP.add)
            nc.scalar.copy(psB_sb[:], psumB[0][:])
            nc.vector.tensor_tensor(psB_sb[:], psB_sb[:], psumB[1][:], OP.add)
            nc.vector.tensor_tensor(scr[:], psB_sb[:, 0:SUB], eye[:], OP.mult)
            nc.vector.tensor_reduce(stats[:, 0:1], scr[:], AX.X, OP.add)   # t.L1c
            nc.vector.tensor_copy(stats[:, 1:2], psB_sb[:, SUB:SUB + 1])   # sum t
            nc.vector.tensor_tensor(scr[:], psB_sb[:, SUB + 1:2 * SUB + 1],
                                    eye[:], OP.mult)
            nc.vector.tensor_reduce(stats[:, 2:3], scr[:], AX.X, OP.add)   # t.L2
            nc.vector.tensor_tensor(scr[:], psB_sb[:, 2 * SUB + 1:3 * SUB + 1],
                                    eye[:], OP.mult)
            nc.vector.tensor_reduce(stats[:, 3:4], scr[:], AX.X, OP.add)   # x.t
            nc.sync.dma_start(stats_d[:], stats[:])

    nc.finalize()
    return nc


def _get_nc():
    if "nc" not in _CACHE:
        _CACHE["nc"] = _build_nc()
    return _CACHE["nc"]


def kernel(inputs: np.ndarray, targets: np.ndarray) -> np.ndarray:
    from concourse.bass_utils import run_bass_kernel_spmd

    xf = np.ascontiguousarray(inputs, dtype=np.float32).reshape(-1)
    tf = np.ascontiguousarray(targets, dtype=np.float32).reshape(-1)
    assert xf.size == TOTAL and tf.size == TOTAL

    eye = np.eye(P, SUB, dtype=np.float32)
    shard = TOTAL // N_CORES
    in_maps = []
    for c in range(N_CORES):
        in_maps.append({
            "x": xf[c * shard:(c + 1) * shard].reshape(P, COLS),
            "t": tf[c * shard:(c + 1) * shard].reshape(P, COLS),
            "eye": eye,
        })

    nc = _get_nc()
    res = run_bass_kernel_spmd(nc, in_maps, core_ids=list(range(N_CORES)))

    s_xt = s_x = s_t = t1 = t2 = s_l2 = 0.0
    n_starts = 0.0
    for c in range(N_CORES):
        stt = res.results[c]["stats"].astype(np.float64)
        t1 += stt[:, 0].sum()
        s_t += stt[:, 1].sum()
        t2 += stt[:, 2].sum()
        s_xt += stt[:, 3].sum()
        s_x += stt[0, 5] + stt[0, 6]       # each row already holds bank totals
        s_l2 += stt[:, 8:8 + NCHUNK].sum()
        n_starts += stt[:, 20:20 + NCHUNK].sum()

    # Host-side boundary run starts: row boundaries (incl. shard cuts) and
    # the first element.  1023 pairs + 1 element - O(1) work.
    prev = xf[COLS - 1:-1:COLS]
    cur = xf[COLS::COLS]
    n_starts += np.count_nonzero((cur != 0) & (prev == 0))
    n_starts += float(xf[0] != 0)

    dice = 1.0 - (2.0 * s_xt + SMOOTH) / (s_x + s_t + SMOOTH)
    bce = -(t1 - t2 + s_l2) / TOTAL
    topo = abs(n_starts - 1.0) / IMAGE_PIXELS
    loss = bce * BCE_WEIGHT + dice + topo * TOPOLOGY_WEIGHT
    return np.array(loss, dtype=np.float32)
